# revision 17
# baseline (speedup 1.0000x reference)
"""BatchHardTripletLoss on 8 Trainium2 NeuronCores.

Strategy (data-parallel over anchor rows, samples pre-sorted by label):
  - host sorts samples by label (loss is permutation-invariant); core c owns
    anchor rows [c*512, (c+1)*512).
  - per-core column ROTATION puts the core's same-label bands at columns
    [0, ~600): each row's positives are a contiguous [lo_i, hi_i) band.
  - PE: psum = -2 e_m.e_n via fp8(e4m3) DoubleRow matmuls only (2x rate,
    D=512 paired as [128, 2, *]; k-major order so consecutive matmuls share
    the stationary operand and LDWEIGHTS pipelines).
  - DVE mining with host-precomputed fp16 "colterm + 8192*band" tiles on
    the second read port (in1):
      hp = max(w + ctmin + (-8192)) over a 256-wide window  (ADD_ADD_MAX)
      hn = min(ADD_MIN group0 w + ctmin;  ADD_MIN group1 w + ctg1)
  - loop is group-phased (all group-0 blocks, then all group-1) so the
    group-1 DMAs stream in behind the group-0 compute.
  - host: min of hn parts, add row terms, sqrt, validity via bincount, mean.
"""

import dataclasses

import numpy as np
import ml_dtypes

import concourse.bacc as bacc
import concourse.mybir as mybir
from concourse.bass_utils import run_bass_kernel_spmd
from concourse.tile import TileContext
from concourse import dve_ops as _dve_ops
from concourse.dve_spec import (
    AluOp, C2, MaxNeg, Spec, Src0, Src1, lower,
)
from concourse.dve_uop import DveOpSpec


def _register_op(name, spec):
    for op in _dve_ops.OPS:
        if op.name == name:
            return op
    op = _dve_ops.DveOp(name, spec, subdim=False, uops_sha={})
    _dve_ops.OPS.append(op)
    opcode = _dve_ops._CUSTOM_DVE_ROW_BASE + len(_dve_ops.OPS) - 1
    assert opcode < 0x20
    _dve_ops._SUB_OPCODE_FOR_NAME[name] = opcode
    _dve_ops.CUSTOM_DVE_SPECS[name] = spec
    shas = {}
    for ver in ("v3", "v4"):
        s = DveOpSpec(name=name, opcode=opcode, uops=lower(spec, ver=ver),
                      rd1_en=True)
        shas[ver] = s.sha(ver)
    op = dataclasses.replace(op, uops_sha=shas)
    _dve_ops.OPS[-1] = op
    return op


ADD_MIN_OP = _register_op(
    "ANT_ADD_MIN",
    Spec(
        body=Src0 + Src1,
        accum=AluOp.MIN,
        accum_init=C2,
        reference=lambda in0, in1, s0, s1, imm2: (in0 + in1).astype(np.float32),
    ),
)

ADD_ADD_MAX_OP = _register_op(
    "ANT_ADD_ADD_MAX",
    Spec(
        body=Src0 + Src1 + C2,
        accum=AluOp.MAX,
        accum_init=MaxNeg,
        reference=lambda in0, in1, s0, s1, imm2: (
            in0 + in1 + np.float32(imm2)).astype(np.float32),
    ),
)

B = 4096          # batch (anchors)
D = 512           # embedding dim
N_CORES = 8
ROWS = B // N_CORES      # 512 anchor rows per core
P = 128                  # partitions
MT = ROWS // P           # 4 m-tiles per core
NW = 512                 # psum bank width (fp32)
GW = 2048                # column group width (4 banks)
NG = B // GW             # 2 column groups
KT = D // P              # 4 contraction k-tiles (paired 2x for DoubleRow)

MARGIN = 0.5
EPS = 1e-6
BIG = 65536.0            # accum_init for the min ops
PEN = 8192.0             # band bump baked into the fp16 in1 tiles
AW = 256                 # band-max window width per m-tile
WLO = [0, 64, 192, 320]  # band-max window start per m-tile (t*128-64, clamped)

_nc_cache = {}


def _build(reps=1):
    nc = bacc.Bacc("TRN2", target_bir_lowering=False)
    fp16 = mybir.dt.float16
    fp8 = mybir.dt.float8e4
    f32 = mybir.dt.float32
    DR = mybir.MatmulPerfMode.DoubleRow

    et = nc.dram_tensor("et", [D, B], fp8, kind="ExternalInput")
    eblk = nc.dram_tensor("eblk", [D, ROWS], fp8, kind="ExternalInput")
    ctmin = nc.dram_tensor("ctmin", [P, MT * GW], fp16, kind="ExternalInput")
    ctg1 = nc.dram_tensor("ctg1", [P, GW], fp16, kind="ExternalInput")
    outd = nc.dram_tensor("out", [reps, P, 3 * MT], f32, kind="ExternalOutput")

    with TileContext(nc) as tc:
        with (
            tc.tile_pool(name="etp", bufs=1) as etp,
            tc.tile_pool(name="ebp", bufs=1) as ebp,
            tc.tile_pool(name="wp", bufs=2) as wp,
            tc.tile_pool(name="accp", bufs=2) as accp,
            tc.tile_pool(name="psp", bufs=2, space="PSUM") as psp,
        ):
            # --- PE warmup: dense tiny matmuls while input DMAs run -------
            warm = etp.tile([P, 64], fp16, tag="warm")
            nc.vector.memset(warm, 0.0)
            wps = psp.tile([P, GW], f32, tag="ps", name="wps")
            for _ in range(96):
                nc.tensor.matmul(wps[:64, 0:64], warm[:, 0:64], warm[:, 0:64],
                                 start=True, stop=True)

            # --- input DMAs: ONE queue, in order of first use -------------
            eb_all = ebp.tile([P, KT * ROWS], fp8, tag="eb", name="eb_all")
            et_all = etp.tile([P, KT * B], fp8, tag="et", name="et_all")
            eb4 = eb_all.rearrange("p (k n) -> p k n", k=KT)
            et4 = et_all.rearrange("p (k n) -> p k n", k=KT)
            ebd4 = eblk.rearrange("(k p) n -> p k n", p=P)
            etd4 = et.rearrange("(k p) n -> p k n", p=P)
            ctmin_sb = etp.tile([P, MT * GW], fp16, tag="ctmin")
            ctg1_sb = etp.tile([P, GW], fp16, tag="ctg1")

            nc.gpsimd.dma_start(out=eb4, in_=ebd4)
            # group-0 et in 4 need-ordered pieces so the PE can stream
            # through the first block's chunks while data arrives
            HG = GW // 2
            for u, a in ((0, 0), (0, HG), (2, 0), (2, HG)):
                nc.gpsimd.dma_start(out=et4[:, u:u + 2, a:a + HG],
                                    in_=etd4[:, u:u + 2, a:a + HG])
            nc.gpsimd.dma_start(out=ctmin_sb[:, 0:GW], in_=ctmin[:, 0:GW])
            nc.gpsimd.dma_start(out=ctmin_sb[:, GW:2 * GW],
                                in_=ctmin[:, GW:2 * GW])
            nc.gpsimd.dma_start(out=et4[:, 0:2, GW:B], in_=etd4[:, 0:2, GW:B])
            nc.gpsimd.dma_start(out=et4[:, 2:4, GW:B], in_=etd4[:, 2:4, GW:B])
            nc.gpsimd.dma_start(out=ctmin_sb[:, 2 * GW:3 * GW],
                                in_=ctmin[:, 2 * GW:3 * GW])
            nc.gpsimd.dma_start(out=ctmin_sb[:, 3 * GW:4 * GW],
                                in_=ctmin[:, 3 * GW:4 * GW])
            nc.gpsimd.dma_start(out=ctg1_sb, in_=ctg1[:, :])

            for r in range(reps):
                osb = accp.tile([P, 3 * MT], f32, tag="osb", name="osb")
                for g in range(NG):
                    for t in range(MT):
                        ms = slice(t * P, (t + 1) * P)
                        ps = psp.tile([P, GW], f32, tag="ps", name="ps")
                        # k-major: consecutive matmuls share the stationary
                        # operand; gram: w = -2 e_m.e_n (two K=256 passes)
                        for u in range(2):
                            for j in range(GW // NW):
                                cs = slice(g * GW + j * NW,
                                           g * GW + (j + 1) * NW)
                                js = slice(j * NW, (j + 1) * NW)
                                nc.tensor.matmul(
                                    ps[:, js],
                                    eb4[:, 2 * u:2 * u + 2, ms],
                                    et4[:, 2 * u:2 * u + 2, cs],
                                    start=(u == 0), stop=(u == 1),
                                    perf_mode=DR,
                                )
                        scr = wp.tile([P, GW], f32, tag="scr", name="scr")
                        if g == 0:
                            # hardest positive: windowed max of w + ct - PEN
                            nc.vector._custom_dve(
                                ADD_ADD_MAX_OP,
                                out=scr[:, 0:AW],
                                in0=ps[:, WLO[t]:WLO[t] + AW],
                                in1=ctmin_sb[:, t * GW + WLO[t]:
                                             t * GW + WLO[t] + AW],
                                imm2=-PEN,
                                accum_out=osb[:, t:t + 1],
                            )
                            # hardest negative, group 0 (band pushed +PEN)
                            nc.vector._custom_dve(
                                ADD_MIN_OP,
                                out=scr,
                                in0=ps,
                                in1=ctmin_sb[:, t * GW:(t + 1) * GW],
                                imm2=BIG,
                                accum_out=osb[:, MT + t:MT + t + 1],
                            )
                        else:
                            # hardest negative, group 1 (no band columns)
                            nc.vector._custom_dve(
                                ADD_MIN_OP,
                                out=scr,
                                in0=ps,
                                in1=ctg1_sb,
                                imm2=BIG,
                                accum_out=osb[:, 2 * MT + t:2 * MT + t + 1],
                            )
                    if g == 0:
                        nc.gpsimd.dma_start(out=outd[r][:, 0:2 * MT],
                                            in_=osb[:, 0:2 * MT])
                nc.gpsimd.dma_start(out=outd[r][:, 2 * MT:3 * MT],
                                    in_=osb[:, 2 * MT:3 * MT])
    nc.compile()
    return nc


def _get_nc(reps=1):
    if reps not in _nc_cache:
        _nc_cache[reps] = _build(reps)
    return _nc_cache[reps]


def _prepare_inputs(embeddings, labels):
    f8 = ml_dtypes.float8_e4m3
    Ef = np.ascontiguousarray(np.asarray(embeddings, dtype=np.float32))
    lab = np.asarray(labels).astype(np.int64)
    perm = np.argsort(lab, kind="stable")
    Ef = Ef[perm]
    labp = lab[perm]

    sq = np.sum(Ef * Ef, axis=1, dtype=np.float32)          # [B]
    s = np.sum(Ef, axis=1, dtype=np.float32)                # [B]
    rowterm = (sq + 2.0 * EPS * s + D * EPS * EPS).astype(np.float32)
    colterm = (sq - 2.0 * EPS * s).astype(np.float32)

    # fp8 embeddings, scaled by sqrt(2) so gram = 2 e.e
    et8 = np.ascontiguousarray(
        (Ef * np.float32(np.sqrt(2.0))).astype(f8).T)          # [D, B]
    en8 = np.ascontiguousarray(
        (Ef * np.float32(-np.sqrt(2.0))).astype(f8).T)         # [D, B]

    seg_start = np.searchsorted(labp, labp, side="left")
    seg_end = np.searchsorted(labp, labp, side="right")

    jj = np.arange(GW)
    in_maps = []
    for c in range(N_CORES):
        r0, r1 = c * ROWS, (c + 1) * ROWS
        w0 = int(seg_start[r0])
        lo_b = (seg_start[r0:r1] - w0).astype(np.int64)
        hi_b = (seg_end[r0:r1] - w0).astype(np.int64)
        colperm = (np.arange(B) + w0) % B
        ctrot = colterm[colperm]

        ctmin_a = np.empty((P, MT * GW), dtype=np.float16)
        for t in range(MT):
            tl = lo_b[t * P:(t + 1) * P][:, None]
            th = hi_b[t * P:(t + 1) * P][:, None]
            assert tl.min() >= WLO[t] and th.max() <= WLO[t] + AW, (
                c, t, tl.min(), th.max())
            band = (jj[None, :] >= tl) & (jj[None, :] < th)      # [P, GW]
            ctmin_a[:, t * GW:(t + 1) * GW] = (
                ctrot[None, 0:GW] + np.float32(PEN) * band)

        in_maps.append({
            "et": np.ascontiguousarray(et8[:, colperm]),
            "eblk": np.ascontiguousarray(en8[:, r0:r1]),
            "ctmin": ctmin_a,
            "ctg1": np.ascontiguousarray(
                np.broadcast_to(ctrot[None, GW:B], (P, GW))
            ).astype(np.float16),
        })
    return in_maps, labp, rowterm


def _postprocess(results, labp, rowterm):
    # out[0]: [P, 3*MT]: hp [0:MT], hn_g0 [MT:2MT], hn_g1 [2MT:3MT]
    hp_l, hn_l = [], []
    for r in results:
        o = r["out"][0]                                   # [P, 3*MT]
        hp_l.append(o[:, 0:MT].T.reshape(-1))
        hn_l.append(np.minimum(o[:, MT:2 * MT], o[:, 2 * MT:3 * MT])
                    .T.reshape(-1))
    hp_raw = np.concatenate(hp_l)
    hn_raw = np.concatenate(hn_l)
    hp2 = hp_raw + rowterm
    hn2 = hn_raw + rowterm
    hp = np.sqrt(np.maximum(hp2, 0.0, dtype=np.float32))
    hn = np.sqrt(np.maximum(hn2, 0.0, dtype=np.float32))

    cnt_lab = np.bincount(labp, minlength=1)
    n_same = cnt_lab[labp]
    valid = (n_same > 1) & (n_same < B)
    per = np.where(valid, np.maximum(hp - hn + np.float32(MARGIN), 0.0), 0.0)
    cnt = np.float32(valid.sum())
    if cnt > 0:
        loss = np.float32(per.sum(dtype=np.float32) / max(cnt, np.float32(1.0)))
    else:
        loss = np.float32(0.0)
    return np.asarray(loss, dtype=np.float32)


def _run(in_maps, reps=1, **kw):
    nc = _get_nc(reps)
    return run_bass_kernel_spmd(nc, in_maps, core_ids=list(range(N_CORES)), **kw)


def kernel(embeddings, labels):
    in_maps, labp, rowterm = _prepare_inputs(embeddings, labels)
    res = _run(in_maps)
    return _postprocess(res.results, labp, rowterm)


# revision 18
# speedup vs baseline: 1.0548x; 1.0548x over previous
"""BatchHardTripletLoss on 8 Trainium2 NeuronCores.

Strategy (data-parallel over anchor rows, samples pre-sorted by label):
  - host sorts samples by label (loss is permutation-invariant); core c owns
    anchor rows [c*512, (c+1)*512).
  - per-core column ROTATION puts the core's same-label bands at columns
    [0, ~600): each row's positives are a contiguous [lo_i, hi_i) band.
  - PE: psum = -2 e_m.e_n via fp8(e4m3) DoubleRow matmuls only (2x rate,
    D=512 paired as [128, 2, *]; k-major order so consecutive matmuls share
    the stationary operand and LDWEIGHTS pipelines).
  - DVE mining with host-precomputed fp16 "colterm + 8192*band" tiles on
    the second read port (in1):
      hp = max(w + ctmin + (-8192)) over a 256-wide window  (ADD_ADD_MAX)
      hn = min(ADD_MIN group0 w + ctmin;  ADD_MIN group1 w + ctg1)
  - loop is group-phased (all group-0 blocks, then all group-1) so the
    group-1 DMAs stream in behind the group-0 compute.
  - host: min of hn parts, add row terms, sqrt, validity via bincount, mean.
"""

import dataclasses

import numpy as np
import ml_dtypes

import concourse.bacc as bacc
import concourse.mybir as mybir
from concourse.bass_utils import run_bass_kernel_spmd
from concourse.tile import TileContext
from concourse import dve_ops as _dve_ops
from concourse.dve_spec import (
    AluOp, C2, MaxNeg, Spec, Src0, Src1, lower,
)
from concourse.dve_uop import DveOpSpec


def _register_op(name, spec):
    for op in _dve_ops.OPS:
        if op.name == name:
            return op
    op = _dve_ops.DveOp(name, spec, subdim=False, uops_sha={})
    _dve_ops.OPS.append(op)
    opcode = _dve_ops._CUSTOM_DVE_ROW_BASE + len(_dve_ops.OPS) - 1
    assert opcode < 0x20
    _dve_ops._SUB_OPCODE_FOR_NAME[name] = opcode
    _dve_ops.CUSTOM_DVE_SPECS[name] = spec
    shas = {}
    for ver in ("v3", "v4"):
        s = DveOpSpec(name=name, opcode=opcode, uops=lower(spec, ver=ver),
                      rd1_en=True)
        shas[ver] = s.sha(ver)
    op = dataclasses.replace(op, uops_sha=shas)
    _dve_ops.OPS[-1] = op
    return op


ADD_MIN_OP = _register_op(
    "ANT_ADD_MIN",
    Spec(
        body=Src0 + Src1,
        accum=AluOp.MIN,
        accum_init=C2,
        reference=lambda in0, in1, s0, s1, imm2: (in0 + in1).astype(np.float32),
    ),
)

ADD_ADD_MAX_OP = _register_op(
    "ANT_ADD_ADD_MAX",
    Spec(
        body=Src0 + Src1 + C2,
        accum=AluOp.MAX,
        accum_init=MaxNeg,
        reference=lambda in0, in1, s0, s1, imm2: (
            in0 + in1 + np.float32(imm2)).astype(np.float32),
    ),
)

B = 4096          # batch (anchors)
D = 512           # embedding dim
N_CORES = 8
ROWS = B // N_CORES      # 512 anchor rows per core
P = 128                  # partitions
MT = ROWS // P           # 4 m-tiles per core
NW = 512                 # psum bank width (fp32)
GW = 2048                # column group width (4 banks)
NG = B // GW             # 2 column groups
KT = D // P              # 4 contraction k-tiles (paired 2x for DoubleRow)

MARGIN = 0.5
EPS = 1e-6
BIG = 65536.0            # accum_init for the min ops
PEN = 8192.0             # band bump baked into the fp16 in1 tiles
AW = 256                 # band-max window width per m-tile
WLO = [0, 64, 192, 320]  # band-max window start per m-tile (t*128-64, clamped)

_nc_cache = {}


def _build(reps=1):
    nc = bacc.Bacc("TRN2", target_bir_lowering=False)
    fp16 = mybir.dt.float16
    fp8 = mybir.dt.float8e4
    f32 = mybir.dt.float32
    DR = mybir.MatmulPerfMode.DoubleRow

    et = nc.dram_tensor("et", [D, B], fp8, kind="ExternalInput")
    eblk = nc.dram_tensor("eblk", [D, ROWS], fp8, kind="ExternalInput")
    ctmin = nc.dram_tensor("ctmin", [P, MT * GW], fp16, kind="ExternalInput")
    ctg1 = nc.dram_tensor("ctg1", [P, GW], fp16, kind="ExternalInput")
    outd = nc.dram_tensor("out", [reps, P, 3 * MT], f32, kind="ExternalOutput")

    with TileContext(nc) as tc:
        with (
            tc.tile_pool(name="etp", bufs=1) as etp,
            tc.tile_pool(name="ebp", bufs=1) as ebp,
            tc.tile_pool(name="wp", bufs=2) as wp,
            tc.tile_pool(name="accp", bufs=2) as accp,
            tc.tile_pool(name="psp", bufs=2, space="PSUM") as psp,
        ):
            # --- PE warmup: dense tiny matmuls while input DMAs run -------
            warm = etp.tile([P, 64], fp16, tag="warm")
            nc.vector.memset(warm, 0.0)
            wps = psp.tile([P, GW], f32, tag="ps", name="wps")
            for _ in range(96):
                nc.tensor.matmul(wps[:64, 0:64], warm[:, 0:64], warm[:, 0:64],
                                 start=True, stop=True)

            # --- input DMAs: ONE queue, in order of first use -------------
            eb_all = ebp.tile([P, KT * ROWS], fp8, tag="eb", name="eb_all")
            et_all = etp.tile([P, KT * B], fp8, tag="et", name="et_all")
            eb4 = eb_all.rearrange("p (k n) -> p k n", k=KT)
            et4 = et_all.rearrange("p (k n) -> p k n", k=KT)
            ebd4 = eblk.rearrange("(k p) n -> p k n", p=P)
            etd4 = et.rearrange("(k p) n -> p k n", p=P)
            ctmin_sb = etp.tile([P, MT * GW], fp16, tag="ctmin")
            ctg1_sb = etp.tile([P, GW], fp16, tag="ctg1")

            nc.gpsimd.dma_start(out=eb4, in_=ebd4)
            nc.gpsimd.dma_start(out=et4[:, 0:2, 0:GW], in_=etd4[:, 0:2, 0:GW])
            nc.gpsimd.dma_start(out=et4[:, 2:4, 0:GW], in_=etd4[:, 2:4, 0:GW])
            for t in range(MT):
                ts_ = slice(t * GW, (t + 1) * GW)
                nc.gpsimd.dma_start(out=ctmin_sb[:, ts_], in_=ctmin[:, ts_])
            nc.gpsimd.dma_start(out=et4[:, 0:2, GW:B], in_=etd4[:, 0:2, GW:B])
            nc.gpsimd.dma_start(out=et4[:, 2:4, GW:B], in_=etd4[:, 2:4, GW:B])
            nc.gpsimd.dma_start(out=ctg1_sb, in_=ctg1[:, :])

            for r in range(reps):
                osb = accp.tile([P, 3 * MT], f32, tag="osb", name="osb")
                for g in range(NG):
                    for t in range(MT):
                        ms = slice(t * P, (t + 1) * P)
                        ps = psp.tile([P, GW], f32, tag="ps", name="ps")
                        # k-major: consecutive matmuls share the stationary
                        # operand; gram: w = -2 e_m.e_n (two K=256 passes)
                        for u in range(2):
                            for j in range(GW // NW):
                                cs = slice(g * GW + j * NW,
                                           g * GW + (j + 1) * NW)
                                js = slice(j * NW, (j + 1) * NW)
                                nc.tensor.matmul(
                                    ps[:, js],
                                    eb4[:, 2 * u:2 * u + 2, ms],
                                    et4[:, 2 * u:2 * u + 2, cs],
                                    start=(u == 0), stop=(u == 1),
                                    perf_mode=DR,
                                )
                        scr = wp.tile([P, GW], f32, tag="scr", name="scr")
                        if g == 0:
                            # hardest positive: windowed max of w + ct - PEN
                            nc.vector._custom_dve(
                                ADD_ADD_MAX_OP,
                                out=scr[:, 0:AW],
                                in0=ps[:, WLO[t]:WLO[t] + AW],
                                in1=ctmin_sb[:, t * GW + WLO[t]:
                                             t * GW + WLO[t] + AW],
                                imm2=-PEN,
                                accum_out=osb[:, t:t + 1],
                            )
                            # hardest negative, group 0 (band pushed +PEN)
                            nc.vector._custom_dve(
                                ADD_MIN_OP,
                                out=scr,
                                in0=ps,
                                in1=ctmin_sb[:, t * GW:(t + 1) * GW],
                                imm2=BIG,
                                accum_out=osb[:, MT + t:MT + t + 1],
                            )
                        else:
                            # hardest negative, group 1 (no band columns)
                            nc.vector._custom_dve(
                                ADD_MIN_OP,
                                out=scr,
                                in0=ps,
                                in1=ctg1_sb,
                                imm2=BIG,
                                accum_out=osb[:, 2 * MT + t:2 * MT + t + 1],
                            )
                    if g == 0:
                        nc.gpsimd.dma_start(out=outd[r][:, 0:2 * MT],
                                            in_=osb[:, 0:2 * MT])
                nc.gpsimd.dma_start(out=outd[r][:, 2 * MT:3 * MT],
                                    in_=osb[:, 2 * MT:3 * MT])
    nc.compile()
    return nc


def _get_nc(reps=1):
    if reps not in _nc_cache:
        _nc_cache[reps] = _build(reps)
    return _nc_cache[reps]


def _prepare_inputs(embeddings, labels):
    f8 = ml_dtypes.float8_e4m3
    Ef = np.ascontiguousarray(np.asarray(embeddings, dtype=np.float32))
    lab = np.asarray(labels).astype(np.int64)
    perm = np.argsort(lab, kind="stable")
    Ef = Ef[perm]
    labp = lab[perm]

    sq = np.sum(Ef * Ef, axis=1, dtype=np.float32)          # [B]
    s = np.sum(Ef, axis=1, dtype=np.float32)                # [B]
    rowterm = (sq + 2.0 * EPS * s + D * EPS * EPS).astype(np.float32)
    colterm = (sq - 2.0 * EPS * s).astype(np.float32)

    # fp8 embeddings, scaled by sqrt(2) so gram = 2 e.e
    et8 = np.ascontiguousarray(
        (Ef * np.float32(np.sqrt(2.0))).astype(f8).T)          # [D, B]
    en8 = np.ascontiguousarray(
        (Ef * np.float32(-np.sqrt(2.0))).astype(f8).T)         # [D, B]

    seg_start = np.searchsorted(labp, labp, side="left")
    seg_end = np.searchsorted(labp, labp, side="right")

    jj = np.arange(GW)
    in_maps = []
    for c in range(N_CORES):
        r0, r1 = c * ROWS, (c + 1) * ROWS
        w0 = int(seg_start[r0])
        lo_b = (seg_start[r0:r1] - w0).astype(np.int64)
        hi_b = (seg_end[r0:r1] - w0).astype(np.int64)
        colperm = (np.arange(B) + w0) % B
        ctrot = colterm[colperm]

        ctmin_a = np.empty((P, MT * GW), dtype=np.float16)
        for t in range(MT):
            tl = lo_b[t * P:(t + 1) * P][:, None]
            th = hi_b[t * P:(t + 1) * P][:, None]
            assert tl.min() >= WLO[t] and th.max() <= WLO[t] + AW, (
                c, t, tl.min(), th.max())
            band = (jj[None, :] >= tl) & (jj[None, :] < th)      # [P, GW]
            ctmin_a[:, t * GW:(t + 1) * GW] = (
                ctrot[None, 0:GW] + np.float32(PEN) * band)

        in_maps.append({
            "et": np.ascontiguousarray(et8[:, colperm]),
            "eblk": np.ascontiguousarray(en8[:, r0:r1]),
            "ctmin": ctmin_a,
            "ctg1": np.ascontiguousarray(
                np.broadcast_to(ctrot[None, GW:B], (P, GW))
            ).astype(np.float16),
        })
    return in_maps, labp, rowterm


def _postprocess(results, labp, rowterm):
    # out[0]: [P, 3*MT]: hp [0:MT], hn_g0 [MT:2MT], hn_g1 [2MT:3MT]
    hp_l, hn_l = [], []
    for r in results:
        o = r["out"][0]                                   # [P, 3*MT]
        hp_l.append(o[:, 0:MT].T.reshape(-1))
        hn_l.append(np.minimum(o[:, MT:2 * MT], o[:, 2 * MT:3 * MT])
                    .T.reshape(-1))
    hp_raw = np.concatenate(hp_l)
    hn_raw = np.concatenate(hn_l)
    hp2 = hp_raw + rowterm
    hn2 = hn_raw + rowterm
    hp = np.sqrt(np.maximum(hp2, 0.0, dtype=np.float32))
    hn = np.sqrt(np.maximum(hn2, 0.0, dtype=np.float32))

    cnt_lab = np.bincount(labp, minlength=1)
    n_same = cnt_lab[labp]
    valid = (n_same > 1) & (n_same < B)
    per = np.where(valid, np.maximum(hp - hn + np.float32(MARGIN), 0.0), 0.0)
    cnt = np.float32(valid.sum())
    if cnt > 0:
        loss = np.float32(per.sum(dtype=np.float32) / max(cnt, np.float32(1.0)))
    else:
        loss = np.float32(0.0)
    return np.asarray(loss, dtype=np.float32)


def _run(in_maps, reps=1, **kw):
    nc = _get_nc(reps)
    return run_bass_kernel_spmd(nc, in_maps, core_ids=list(range(N_CORES)), **kw)


def kernel(embeddings, labels):
    in_maps, labp, rowterm = _prepare_inputs(embeddings, labels)
    res = _run(in_maps)
    return _postprocess(res.results, labp, rowterm)
